# revision 30
# baseline (speedup 1.0000x reference)
"""Trainium2 Bass kernel for nn_CRAU (per-channel sparse attention).

Computation (per batch b, channel c):
  qc  = Wq @ src (1x1 conv; bq folded into the S-reduction seed)
  S[c,t] = sum_d unfold(qc)[c,t,d] * feat[c,d] * (1/64)      t in 3x3 window
  A   = softmax_t(S)
  vc  = Wv @ feat + bv (1x1 conv)
  out = fold(A outer vc) * src

Sharding: 8 cores = 4 batches x 2 output-channel groups of 128. The
attention is fully per-channel, so with channel-group sharding each core
owns the complete spatial reduction for its channels -- no collectives.
Each core's inputs are channel-permuted host-side (own group first) so the
SPMD program always works on partition rows 0..127.

Layout: all tensors f16 host-side; the padded src slab is packed as 4
row/col-parity quadrant planes [EE|EO|OE|OO] so the 9-offset q.k reduce,
the fold, and the final multiply all read unit-stride blocks. qc inherits
the quadrant layout from the matmul rhs. The fold runs on the PE as
diag(A_t) matmuls accumulating parity half-planes in PSUM; finals
(plane * src) are split between DVE (PSUM-direct) and GpSimd.
"""

import numpy as np

N_CORES = 8
SRC_R, SRC_C = 129, 129         # per-core padded src slab (full spatial)
FEAT_R, FEAT_C = 65, 66         # padded feat slab
# quadrant plane offsets in the packed src/qc layout
O_EE = 0
O_EO = O_EE + 65 * 65           # 4225
O_OE = O_EO + 65 * 64           # 8385
O_OO = O_OE + 64 * 65           # 12545
SRCN = O_OO + 64 * 64           # 16641
FEATN = FEAT_R * FEAT_C         # 4290
OUTN = 4 * 4096                 # 4 parity planes per channel
SCALE = 1.0 / 64.0

_prog_cache = {}
TRACE = False
TRACE_KW = {}
LAST_RESULT = [None]


def _build(add_bv: bool, add_bq: bool = False):
    import concourse.mybir as mybir
    import concourse.tile as tile
    from concourse import bacc
    from concourse.dve_ops import TENSOR_TENSOR_REDUCE

    f32 = mybir.dt.float32
    f16 = mybir.dt.float16
    ADD = mybir.AluOpType.add
    MULT = mybir.AluOpType.mult
    MAX = mybir.AluOpType.max
    AX = mybir.AxisListType.X
    Exp = mybir.ActivationFunctionType.Exp
    Copy = mybir.ActivationFunctionType.Copy

    nc = bacc.Bacc("TRN2", target_bir_lowering=False, debug=False,
                   num_devices=N_CORES)

    src_d = nc.dram_tensor("src", [256, SRCN], f16, kind="ExternalInput").ap()
    feat_d = nc.dram_tensor("feat", [256, FEATN], f16, kind="ExternalInput").ap()
    wpack_d = nc.dram_tensor("wpack", [256, 256], f16, kind="ExternalInput").ap()
    eye_d = nc.dram_tensor("eye", [128, 128], f16, kind="ExternalInput").ap()
    sinit_d = nc.dram_tensor("s_init", [128, 9], f32, kind="ExternalInput").ap()
    bv_d = nc.dram_tensor("bv", [128, 1], f32, kind="ExternalInput").ap()
    out_d = nc.dram_tensor("out", [128, OUTN], f16, kind="ExternalOutput").ap()
    evs_d = nc.dram_tensor("evs", [128, 9], f32, kind="ExternalOutput").ap()

    QCH = 2048
    q_chunks = []
    c0 = 0
    while c0 < SRCN:
        q_chunks.append((c0, min(QCH, SRCN - c0)))
        c0 += QCH

    with tile.TileContext(nc) as tc:
        with (
            tc.tile_pool(name="srcp", bufs=2) as srcp,
            tc.tile_pool(name="featp", bufs=2) as featp,
            tc.tile_pool(name="qcp", bufs=1) as qcp,
            tc.tile_pool(name="vcp", bufs=1) as vcp,
            tc.tile_pool(name="constp", bufs=2) as constp,
            tc.tile_pool(name="smp", bufs=1) as smp,
            tc.tile_pool(name="scrp", bufs=2) as scrp,
            tc.tile_pool(name="outp", bufs=4) as outp,
            tc.tile_pool(name="ps", bufs=2, space="PSUM") as ps,
        ):
            # ---- loads ----
            w_t = []
            for kt in range(2):
                wt = constp.tile([128, 256], f16, tag="w", name=f"w{kt}")
                nc.sync.dma_start(wt[:], wpack_d[128 * kt:128 * kt + 128, :])
                w_t.append(wt)
            eye = constp.tile([128, 128], f16, tag="eye")
            nc.sync.dma_start(eye[:], eye_d[:, :])

            # smalls: [0:9] S [9:18] Ev [29] sum [30] r | [32:41] sinit
            # [41:42] bv
            sm = smp.tile([128, 48], f32, tag="smalls")
            nc.scalar.dma_start(sm[:, 32:41], sinit_d[:, :])
            if add_bv:
                nc.scalar.dma_start(sm[:, 41:42], bv_d[:, :])

            src_t = [srcp.tile([128, SRCN], f16, tag="src", name=f"src{kt}")
                     for kt in range(2)]
            feat_t = [featp.tile([128, FEATN], f16, tag="feat",
                                 name=f"feat{kt}") for kt in range(2)]
            # single gpsimd DMA stream, ordered so the reduce-critical data
            # lands first: src chunks 0-2, then k rows (feat_t[0] top), then
            # the rest of src, then the remaining feat
            HALF_F = 33 * FEAT_C
            for ci, (c0, csz) in enumerate(q_chunks):
                for kt in range(2):
                    nc.gpsimd.dma_start(
                        src_t[kt][:, c0:c0 + csz],
                        src_d[128 * kt:128 * kt + 128, c0:c0 + csz])
                if ci == 2:
                    nc.gpsimd.dma_start(feat_t[0][:, 0:HALF_F],
                                        feat_d[0:128, 0:HALF_F])
                elif ci == 3:
                    nc.gpsimd.dma_start(feat_t[0][:, HALF_F:FEATN],
                                        feat_d[0:128, HALF_F:FEATN])
            nc.gpsimd.dma_start(feat_t[1][:], feat_d[128:256, :])

            # ---- q-conv (PE) + PSUM->SBUF f16 copies (ScalarE) ----
            qct = qcp.tile([128, SRCN], f16, tag="qc")
            qEE = qct[:, O_EE:O_EO].rearrange("p (r q) -> p r q", q=65)
            qEO = qct[:, O_EO:O_OE].rearrange("p (r q) -> p r q", q=64)
            qOE = qct[:, O_OE:O_OO].rearrange("p (r q) -> p r q", q=65)
            qOO = qct[:, O_OO:SRCN].rearrange("p (r q) -> p r q", q=64)
            k3 = feat_t[0].rearrange("p (r q) -> p r q", q=FEAT_C)
            k64 = k3[:, 0:64, 0:64]

            def qconv_chunk(c0, csz):
                pt = ps.tile([128, 2048], f32, tag="mm", name="pt")
                for kt in range(2):
                    for s0 in range(0, csz, 512):
                        ssz = min(512, csz - s0)
                        nc.tensor.matmul(
                            pt[:, s0:s0 + ssz],
                            lhsT=w_t[kt][:, 0:128],
                            rhs=src_t[kt][:, c0 + s0:c0 + s0 + ssz],
                            start=(kt == 0), stop=(kt == 1))
                nc.scalar.copy(qct[:, c0:c0 + csz], pt[:, 0:csz])

            # offloaded EE offsets: DVE 2x products + ScalarE accum-reduce.
            # (only when bq==0 -- the accum path has no seed slot)
            OFFL = [] if add_bq else [(0, 0), (0, 2), (2, 0)]

            def emit_product(i, j):
                prod = scrp.tile([128, 4096], f16, tag="prod", name="prod")
                in0 = qEE[:, (1 if i == 2 else 0):(65 if i == 2 else 64),
                          (1 if j == 2 else 0):(65 if j == 2 else 64)]
                prod3 = prod.rearrange("p (r q) -> p r q", q=64)
                nc.vector.tensor_tensor(out=prod3[:], in0=in0, in1=k64,
                                        op=MULT)
                return prod

            def emit_accred(t, prod):
                dump = scrp.tile([128, 4096], f16, tag="dump")
                nc.scalar.activation(dump[:], prod[:], Copy, bias=0.0,
                                     scale=SCALE, accum_out=sm[:, t:t + 1])

            done = {}

            def emit_fused(i, j, ra=0, rb=64):
                t = 3 * i + j
                rlo = (1 if i == 2 else 0) + ra
                nrow = rb - ra
                if i == 1:
                    plane = qOO if j == 1 else qOE
                else:
                    plane = qEO if j == 1 else qEE
                clo = 1 if j == 2 else 0
                in0 = plane[:, rlo:rlo + nrow, clo:clo + 64]
                seed = sm[:, 32 + t:33 + t] if t not in done \
                    else sm[:, t:t + 1]
                scr = scrp.tile([128, 4096], f16, tag="scr")
                scr3 = scr.rearrange("p (r q) -> p r q", q=64)
                nc.vector._custom_dve(
                    TENSOR_TENSOR_REDUCE,
                    out=scr3[:, 0:nrow, :], in0=in0, in1=k64[:, ra:rb, :],
                    s0=seed, s1=SCALE,
                    accum_out=sm[:, t:t + 1])
                done[t] = done.get(t, 0) + 1

            dg = constp.tile([128, 9 * 128], f16, tag="dg")

            def emit_exp(t):
                nc.scalar.activation(sm[:, 9 + t:10 + t], sm[:, t:t + 1],
                                     Exp, bias=0.0, scale=1.0)

            def emit_diag(t):
                nc.vector.tensor_scalar(
                    out=dg[:, 128 * t:128 * t + 128], in0=eye[:],
                    scalar1=sm[:, 9 + t:10 + t], scalar2=None, op0=MULT)

            vt = vcp.tile([128, FEATN], f16, tag="vc")

            def emit_vconv():
                for c0 in range(0, FEATN, 2048):
                    csz = min(2048, FEATN - c0)
                    pt = ps.tile([128, 2048], f32, tag="mm", name="ptv")
                    for kt in range(2):
                        for s0 in range(0, csz, 512):
                            ssz = min(512, csz - s0)
                            nc.tensor.matmul(
                                pt[:, s0:s0 + ssz],
                                lhsT=w_t[kt][:, 128:256],
                                rhs=feat_t[kt][:, c0 + s0:c0 + s0 + ssz],
                                start=(kt == 0), stop=(kt == 1))
                    if add_bv:
                        nc.vector.tensor_scalar(
                            out=vt[:, c0:c0 + csz], in0=pt[:, 0:csz],
                            scalar1=sm[:, 41:42], scalar2=None, op0=ADD)
                    else:
                        nc.scalar.copy(vt[:, c0:c0 + csz], pt[:, 0:csz])
                if add_bv:
                    v3m = vt.rearrange("p (r q) -> p r q", q=FEAT_C)
                    nc.gpsimd.memset(v3m[:, FEAT_R - 1, :], 0.0)
                    nc.gpsimd.memset(v3m[:, :, 64:66], 0.0)

            # ---- fold plumbing: unnormalized planes; host applies 1/sum ----
            vc3 = vt.rearrange("p (r q) -> p r q", q=FEAT_C)
            sEE = src_t[0][:, O_EE:O_EO].rearrange("p (r q) -> p r q", q=65)
            sEO = src_t[0][:, O_EO:O_OE].rearrange("p (r q) -> p r q", q=64)
            sOE = src_t[0][:, O_OE:O_OO].rearrange("p (r q) -> p r q", q=65)
            sOO = src_t[0][:, O_OO:SRCN].rearrange("p (r q) -> p r q", q=64)
            # plane id -> (terms [(t, dr, dc)], src plane, src shift, out col)
            FOLD = {
                "ee": ([(4, 0, 0)], sOO, (0, 0), 0),
                "eo": ([(3, 0, 1), (5, 0, 0)], sOE, (0, 1), 4096),
                "oe": ([(1, 1, 0), (7, 0, 0)], sEO, (1, 0), 8192),
                "oo": ([(0, 1, 1), (2, 1, 0), (6, 0, 1), (8, 0, 0)],
                       sEE, (1, 1), 12288),
            }
            fold_pt = {}

            def fold_mm(pn, hh):
                terms, _, _, _ = FOLD[pn]
                pt = ps.tile([128, 2048], f32, tag="mm", name="ptf")
                pt3 = pt.rearrange("p (r q) -> p r q", q=64)
                nterm = len(terms)
                for n, (t, dr, dc) in enumerate(terms):
                    for s0 in range(4):
                        r0 = 32 * hh + dr + 8 * s0
                        nc.tensor.matmul(
                            pt3[:, 8 * s0:8 * s0 + 8, :],
                            lhsT=dg[:, 128 * t:128 * t + 128],
                            rhs=vc3[:, r0:r0 + 8, dc:dc + 64],
                            start=(n == 0), stop=(n == nterm - 1))
                fold_pt[(pn, hh)] = pt

            def fold_final(pn, hh, eng):
                _, spl, (sro, sco), oc = FOLD[pn]
                pt = fold_pt[(pn, hh)]
                pt3 = pt.rearrange("p (r q) -> p r q", q=64)
                ssl = spl[:, 32 * hh + sro:32 * hh + sro + 32, sco:sco + 64]
                ot = outp.tile([128, 2048], f16, tag="O")
                ot3 = ot.rearrange("p (r q) -> p r q", q=64)
                if eng == "dve":
                    nc.vector.tensor_tensor(out=ot3[:], in0=pt3[:], in1=ssl,
                                            op=MULT)
                else:
                    pf = scrp.tile([128, 2048], f16, tag="pf")
                    nc.scalar.copy(pf[:], pt[:, 0:2048])
                    pf3 = pf.rearrange("p (r q) -> p r q", q=64)
                    nc.gpsimd.tensor_tensor(out=ot3[:], in0=pf3[:],
                                            in1=ssl, op=MULT)
                nc.sync.dma_start(out_d[:, oc + 2048 * hh:
                                        oc + 2048 * hh + 2048], ot[:])

            # ---- the schedule ----
            if not add_bq:
                for ci in range(5):
                    qconv_chunk(*q_chunks[ci])
                emit_fused(2, 2)                    # t8 (EE)
                prods = [emit_product(i, j) for (i, j) in OFFL]
                emit_exp(8)
                emit_accred(0, prods[0])
                emit_exp(0)
                emit_fused(0, 1)                    # t1 (EO)
                qconv_chunk(*q_chunks[5])
                emit_accred(2, prods[1])
                emit_exp(2)
                emit_fused(2, 1)                    # t7 (EO)
                qconv_chunk(*q_chunks[6])
                emit_accred(6, prods[2])
                emit_exp(6)
                for t in (8, 0, 2):
                    emit_diag(t)
                emit_fused(1, 0)                    # t3 (OE)
                emit_exp(1)
                emit_exp(7)
                emit_diag(6)
                emit_diag(1)
                emit_diag(7)
                emit_vconv()
                qconv_chunk(*q_chunks[7])
                qconv_chunk(*q_chunks[8])
                fold_mm("oo", 0)
                fold_mm("oo", 1)
                emit_fused(1, 2)                    # t5 (OE)
                emit_exp(3)
                emit_diag(3)
                fold_final("oo", 0, "dve")
                emit_fused(1, 1)                    # t4 (OO)
                emit_exp(5)
                emit_diag(5)
                fold_final("oo", 1, "dve")
                emit_exp(4)
                emit_diag(4)
                nc.sync.dma_start(evs_d[:, :], sm[:, 9:18])
                fold_mm("oe", 0)
                fold_mm("oe", 1)
                fold_final("oe", 0, "gp")
                fold_final("oe", 1, "gp")
                fold_mm("eo", 0)
                fold_mm("eo", 1)
                fold_final("eo", 0, "dve")
                fold_final("eo", 1, "dve")
                fold_mm("ee", 0)
                fold_mm("ee", 1)
                fold_final("ee", 0, "dve")
                fold_final("ee", 1, "dve")
            else:
                for c0, csz in q_chunks:
                    qconv_chunk(c0, csz)
                emit_vconv()
                for (i, j, ra, rb) in [(0, 0, 0, 32), (0, 2, 0, 32),
                                       (0, 0, 32, 64), (0, 2, 32, 64),
                                       (2, 0, 0, 64), (2, 2, 0, 64),
                                       (0, 1, 0, 64), (2, 1, 0, 64),
                                       (1, 0, 0, 64), (1, 2, 0, 64),
                                       (1, 1, 0, 64)]:
                    emit_fused(i, j, ra, rb)
                nc.scalar.activation(sm[:, 9:18], sm[:, 0:9], Exp,
                                     bias=0.0, scale=1.0)
                nc.sync.dma_start(evs_d[:, :], sm[:, 9:18])
                for t in range(9):
                    emit_diag(t)
                for pn in ("oo", "oe", "eo", "ee"):
                    for hh in range(2):
                        fold_mm(pn, hh)
                        fold_final(pn, hh,
                                   "gp" if pn in ("oe", "eo") else "dve")

    nc.compile()
    return nc


def _get_program(add_bv: bool, add_bq: bool):
    key = (add_bv, add_bq)
    if key not in _prog_cache:
        _prog_cache[key] = _build(add_bv, add_bq)
    return _prog_cache[key]


def _quad_pack(slab):
    """[C, 129, 129] -> [C, 16641] quadrant-packed [EE|EO|OE|OO]."""
    C = slab.shape[0]
    return np.concatenate([
        slab[:, 0::2, 0::2].reshape(C, -1),
        slab[:, 0::2, 1::2].reshape(C, -1),
        slab[:, 1::2, 0::2].reshape(C, -1),
        slab[:, 1::2, 1::2].reshape(C, -1),
    ], axis=1)


def kernel(feat, src, Wq, bq, Wv, bv):
    from concourse.bass_utils import run_bass_kernel_spmd

    feat = np.asarray(feat, dtype=np.float32)
    src = np.asarray(src, dtype=np.float32)
    Wq = np.asarray(Wq, dtype=np.float32)
    bq = np.asarray(bq, dtype=np.float32)
    Wv = np.asarray(Wv, dtype=np.float32)
    bv = np.asarray(bv, dtype=np.float32)
    B, C, H, W = src.shape

    src16 = np.pad(src, ((0, 0), (0, 0), (1, 1), (1, 1))).astype(np.float16)
    feat16 = np.pad(feat, ((0, 0), (0, 0), (0, 1), (0, 2))).astype(np.float16)
    eye = np.eye(128, dtype=np.float16)

    add_bv = bool(np.any(bv))
    have_bq = bool(np.any(bq))
    nc = _get_program(add_bv, have_bq)
    zero_sinit = np.zeros((128, 9), np.float32)

    in_maps = []
    perms = []
    for core in range(N_CORES):
        b, g = core // 2, core % 2
        mine = slice(128 * g, 128 * g + 128)
        other = slice(128 * (1 - g), 128 * (1 - g) + 128)
        perm = np.r_[np.arange(128 * g, 128 * g + 128),
                     np.arange(128 * (1 - g), 128 * (1 - g) + 128)]
        perms.append((b, mine))
        src_slab = _quad_pack(src16[b][perm][:, :SRC_R, :SRC_C])
        feat_slab = feat16[b][perm].reshape(C, FEATN)
        wp = np.concatenate([Wq.T[:, mine], Wv.T[:, mine]], axis=1)[perm]
        if have_bq:
            k = feat[b, mine]
            corr = np.zeros((128, 9), np.float32)
            for i in range(3):
                for j in range(3):
                    valid = np.ones((64, 64), bool)
                    if i == 0:
                        valid[0, :] = False
                    if j == 0:
                        valid[:, 0] = False
                    corr[:, 3 * i + j] = \
                        bq[mine] * (k * valid).sum((1, 2)) * SCALE
            sinit = corr
        else:
            sinit = zero_sinit
        in_maps.append({
            "src": np.ascontiguousarray(src_slab),
            "feat": np.ascontiguousarray(feat_slab),
            "wpack": np.ascontiguousarray(wp.astype(np.float16)),
            "eye": eye,
            "s_init": sinit,
            "bv": bv[mine].reshape(128, 1),
        })

    res = run_bass_kernel_spmd(nc, in_maps, list(range(N_CORES)),
                               trace=TRACE, **TRACE_KW)
    LAST_RESULT[0] = res

    out = np.empty((B, C, H, W), np.float32)
    for core in range(N_CORES):
        b, mine = perms[core]
        r = 1.0 / res.results[core]["evs"].sum(axis=1)
        o = res.results[core]["out"].astype(np.float32).reshape(128, 4, 64, 64)
        o = o * r[:, None, None, None]
        out[b, mine, 0::2, 0::2] = o[:, 0]
        out[b, mine, 0::2, 1::2] = o[:, 1]
        out[b, mine, 1::2, 0::2] = o[:, 2]
        out[b, mine, 1::2, 1::2] = o[:, 3]
    return out


# revision 31
# speedup vs baseline: 1.0286x; 1.0286x over previous
"""Trainium2 Bass kernel for nn_CRAU (per-channel sparse attention).

Computation (per batch b, channel c):
  qc  = Wq @ src (1x1 conv; bq folded into the S-reduction seed)
  S[c,t] = sum_d unfold(qc)[c,t,d] * feat[c,d] * (1/64)      t in 3x3 window
  A   = softmax_t(S)
  vc  = Wv @ feat + bv (1x1 conv)
  out = fold(A outer vc) * src

Sharding: 8 cores = 4 batches x 2 output-channel groups of 128. The
attention is fully per-channel, so with channel-group sharding each core
owns the complete spatial reduction for its channels -- no collectives.
Each core's inputs are channel-permuted host-side (own group first) so the
SPMD program always works on partition rows 0..127.

Layout: all tensors f16 host-side; the padded src slab is packed as 4
row/col-parity quadrant planes [EE|EO|OE|OO] so the 9-offset q.k reduce,
the fold, and the final multiply all read unit-stride blocks. qc inherits
the quadrant layout from the matmul rhs. The fold runs on the PE as
diag(A_t) matmuls accumulating parity half-planes in PSUM; finals
(plane * src) are split between DVE (PSUM-direct) and GpSimd.
"""

import numpy as np

N_CORES = 8
SRC_R, SRC_C = 129, 129         # per-core padded src slab (full spatial)
FEAT_R, FEAT_C = 65, 66         # padded feat slab
# quadrant plane offsets in the packed src/qc layout
O_EE = 0
O_EO = O_EE + 65 * 65           # 4225
O_OE = O_EO + 65 * 64           # 8385
O_OO = O_OE + 64 * 65           # 12545
SRCN = O_OO + 64 * 64           # 16641
FEATN = FEAT_R * FEAT_C         # 4290
OUTN = 4 * 4096                 # 4 parity planes per channel
SCALE = 1.0 / 64.0

_prog_cache = {}
TRACE = False
TRACE_KW = {}
LAST_RESULT = [None]


def _build(add_bv: bool, add_bq: bool = False):
    import concourse.mybir as mybir
    import concourse.tile as tile
    from concourse import bacc
    from concourse.dve_ops import TENSOR_TENSOR_REDUCE

    f32 = mybir.dt.float32
    f16 = mybir.dt.float16
    ADD = mybir.AluOpType.add
    MULT = mybir.AluOpType.mult
    MAX = mybir.AluOpType.max
    AX = mybir.AxisListType.X
    Exp = mybir.ActivationFunctionType.Exp
    Copy = mybir.ActivationFunctionType.Copy

    nc = bacc.Bacc("TRN2", target_bir_lowering=False, debug=False,
                   num_devices=N_CORES)

    src_d = nc.dram_tensor("src", [256, SRCN], f16, kind="ExternalInput").ap()
    feat_d = nc.dram_tensor("feat", [256, FEATN], f16, kind="ExternalInput").ap()
    wpack_d = nc.dram_tensor("wpack", [256, 256], f16, kind="ExternalInput").ap()
    eye_d = nc.dram_tensor("eye", [128, 128], f16, kind="ExternalInput").ap()
    sinit_d = nc.dram_tensor("s_init", [128, 9], f32, kind="ExternalInput").ap()
    bv_d = nc.dram_tensor("bv", [128, 1], f32, kind="ExternalInput").ap()
    out_d = nc.dram_tensor("out", [128, OUTN], f16, kind="ExternalOutput").ap()
    evs_d = nc.dram_tensor("evs", [128, 9], f32, kind="ExternalOutput").ap()

    QCH = 2048
    q_chunks = []
    c0 = 0
    while c0 < SRCN:
        q_chunks.append((c0, min(QCH, SRCN - c0)))
        c0 += QCH

    with tile.TileContext(nc) as tc:
        with (
            tc.tile_pool(name="srcp", bufs=2) as srcp,
            tc.tile_pool(name="featp", bufs=2) as featp,
            tc.tile_pool(name="qcp", bufs=1) as qcp,
            tc.tile_pool(name="vcp", bufs=1) as vcp,
            tc.tile_pool(name="constp", bufs=2) as constp,
            tc.tile_pool(name="smp", bufs=1) as smp,
            tc.tile_pool(name="scrp", bufs=2) as scrp,
            tc.tile_pool(name="outp", bufs=4) as outp,
            tc.tile_pool(name="ps", bufs=2, space="PSUM") as ps,
        ):
            # ---- loads ----
            w_t = []
            for kt in range(2):
                wt = constp.tile([128, 256], f16, tag="w", name=f"w{kt}")
                nc.sync.dma_start(wt[:], wpack_d[128 * kt:128 * kt + 128, :])
                w_t.append(wt)
            eye = constp.tile([128, 128], f16, tag="eye")
            nc.sync.dma_start(eye[:], eye_d[:, :])

            # smalls: [0:9] S [9:18] Ev [29] sum [30] r | [32:41] sinit
            # [41:42] bv
            sm = smp.tile([128, 48], f32, tag="smalls")
            nc.scalar.dma_start(sm[:, 32:41], sinit_d[:, :])
            if add_bv:
                nc.scalar.dma_start(sm[:, 41:42], bv_d[:, :])

            src_t = [srcp.tile([128, SRCN], f16, tag="src", name=f"src{kt}")
                     for kt in range(2)]
            feat_t = [featp.tile([128, FEATN], f16, tag="feat",
                                 name=f"feat{kt}") for kt in range(2)]
            # single gpsimd DMA stream, ordered so the reduce-critical data
            # lands first: src chunks 0-2, then k rows (feat_t[0] top), then
            # the rest of src, then the remaining feat
            HALF_F = 33 * FEAT_C
            for ci, (c0, csz) in enumerate(q_chunks):
                for kt in range(2):
                    nc.gpsimd.dma_start(
                        src_t[kt][:, c0:c0 + csz],
                        src_d[128 * kt:128 * kt + 128, c0:c0 + csz])
                if ci == 2:
                    nc.gpsimd.dma_start(feat_t[0][:, 0:HALF_F],
                                        feat_d[0:128, 0:HALF_F])
                elif ci == 3:
                    nc.gpsimd.dma_start(feat_t[0][:, HALF_F:FEATN],
                                        feat_d[0:128, HALF_F:FEATN])
            nc.gpsimd.dma_start(feat_t[1][:], feat_d[128:256, :])

            # ---- q-conv (PE) + PSUM->SBUF f16 copies (ScalarE) ----
            qct = qcp.tile([128, SRCN], f16, tag="qc")
            qEE = qct[:, O_EE:O_EO].rearrange("p (r q) -> p r q", q=65)
            qEO = qct[:, O_EO:O_OE].rearrange("p (r q) -> p r q", q=64)
            qOE = qct[:, O_OE:O_OO].rearrange("p (r q) -> p r q", q=65)
            qOO = qct[:, O_OO:SRCN].rearrange("p (r q) -> p r q", q=64)
            k3 = feat_t[0].rearrange("p (r q) -> p r q", q=FEAT_C)
            k64 = k3[:, 0:64, 0:64]

            def qconv_chunk(c0, csz):
                pt = ps.tile([128, 2048], f32, tag="mm", name="pt")
                for kt in range(2):
                    for s0 in range(0, csz, 512):
                        ssz = min(512, csz - s0)
                        nc.tensor.matmul(
                            pt[:, s0:s0 + ssz],
                            lhsT=w_t[kt][:, 0:128],
                            rhs=src_t[kt][:, c0 + s0:c0 + s0 + ssz],
                            start=(kt == 0), stop=(kt == 1))
                nc.scalar.copy(qct[:, c0:c0 + csz], pt[:, 0:csz])

            # offloaded EE offsets: DVE 2x products + ScalarE accum-reduce.
            # (only when bq==0 -- the accum path has no seed slot)
            OFFL = [] if add_bq else [(0, 0), (0, 2), (2, 0)]

            def emit_product(i, j):
                prod = scrp.tile([128, 4096], f16, tag="prod", name="prod")
                in0 = qEE[:, (1 if i == 2 else 0):(65 if i == 2 else 64),
                          (1 if j == 2 else 0):(65 if j == 2 else 64)]
                prod3 = prod.rearrange("p (r q) -> p r q", q=64)
                nc.vector.tensor_tensor(out=prod3[:], in0=in0, in1=k64,
                                        op=MULT)
                return prod

            def emit_accred(t, prod):
                dump = scrp.tile([128, 4096], f16, tag="dump")
                nc.scalar.activation(dump[:], prod[:], Copy, bias=0.0,
                                     scale=SCALE, accum_out=sm[:, t:t + 1])

            done = {}

            def emit_fused(i, j, ra=0, rb=64):
                t = 3 * i + j
                rlo = (1 if i == 2 else 0) + ra
                nrow = rb - ra
                if i == 1:
                    plane = qOO if j == 1 else qOE
                else:
                    plane = qEO if j == 1 else qEE
                clo = 1 if j == 2 else 0
                in0 = plane[:, rlo:rlo + nrow, clo:clo + 64]
                seed = sm[:, 32 + t:33 + t] if t not in done \
                    else sm[:, t:t + 1]
                scr = scrp.tile([128, 4096], f16, tag="scr")
                scr3 = scr.rearrange("p (r q) -> p r q", q=64)
                nc.vector._custom_dve(
                    TENSOR_TENSOR_REDUCE,
                    out=scr3[:, 0:nrow, :], in0=in0, in1=k64[:, ra:rb, :],
                    s0=seed, s1=SCALE,
                    accum_out=sm[:, t:t + 1])
                done[t] = done.get(t, 0) + 1

            dg = constp.tile([128, 9 * 128], f16, tag="dg")

            def emit_exp(t):
                nc.scalar.activation(sm[:, 9 + t:10 + t], sm[:, t:t + 1],
                                     Exp, bias=0.0, scale=1.0)

            def emit_diag(t):
                nc.vector.tensor_scalar(
                    out=dg[:, 128 * t:128 * t + 128], in0=eye[:],
                    scalar1=sm[:, 9 + t:10 + t], scalar2=None, op0=MULT)

            vt = vcp.tile([128, FEATN], f16, tag="vc")

            def emit_vconv():
                for c0 in range(0, FEATN, 2048):
                    csz = min(2048, FEATN - c0)
                    pt = ps.tile([128, 2048], f32, tag="mm", name="ptv")
                    for kt in range(2):
                        for s0 in range(0, csz, 512):
                            ssz = min(512, csz - s0)
                            nc.tensor.matmul(
                                pt[:, s0:s0 + ssz],
                                lhsT=w_t[kt][:, 128:256],
                                rhs=feat_t[kt][:, c0 + s0:c0 + s0 + ssz],
                                start=(kt == 0), stop=(kt == 1))
                    if add_bv:
                        nc.vector.tensor_scalar(
                            out=vt[:, c0:c0 + csz], in0=pt[:, 0:csz],
                            scalar1=sm[:, 41:42], scalar2=None, op0=ADD)
                    else:
                        nc.scalar.copy(vt[:, c0:c0 + csz], pt[:, 0:csz])
                if add_bv:
                    v3m = vt.rearrange("p (r q) -> p r q", q=FEAT_C)
                    nc.gpsimd.memset(v3m[:, FEAT_R - 1, :], 0.0)
                    nc.gpsimd.memset(v3m[:, :, 64:66], 0.0)

            # ---- fold plumbing: unnormalized planes; host applies 1/sum ----
            vc3 = vt.rearrange("p (r q) -> p r q", q=FEAT_C)
            sEE = src_t[0][:, O_EE:O_EO].rearrange("p (r q) -> p r q", q=65)
            sEO = src_t[0][:, O_EO:O_OE].rearrange("p (r q) -> p r q", q=64)
            sOE = src_t[0][:, O_OE:O_OO].rearrange("p (r q) -> p r q", q=65)
            sOO = src_t[0][:, O_OO:SRCN].rearrange("p (r q) -> p r q", q=64)
            # plane id -> (terms [(t, dr, dc)], src plane, src shift, out col)
            FOLD = {
                "ee": ([(4, 0, 0)], sOO, (0, 0), 0),
                "eo": ([(3, 0, 1), (5, 0, 0)], sOE, (0, 1), 4096),
                "oe": ([(1, 1, 0), (7, 0, 0)], sEO, (1, 0), 8192),
                "oo": ([(0, 1, 1), (2, 1, 0), (6, 0, 1), (8, 0, 0)],
                       sEE, (1, 1), 12288),
            }
            fold_pt = {}

            def fold_mm(pn, hh):
                terms, _, _, _ = FOLD[pn]
                pt = ps.tile([128, 2048], f32, tag="mm", name="ptf")
                pt3 = pt.rearrange("p (r q) -> p r q", q=64)
                nterm = len(terms)
                for n, (t, dr, dc) in enumerate(terms):
                    for s0 in range(4):
                        r0 = 32 * hh + dr + 8 * s0
                        nc.tensor.matmul(
                            pt3[:, 8 * s0:8 * s0 + 8, :],
                            lhsT=dg[:, 128 * t:128 * t + 128],
                            rhs=vc3[:, r0:r0 + 8, dc:dc + 64],
                            start=(n == 0), stop=(n == nterm - 1))
                fold_pt[(pn, hh)] = pt

            def fold_final(pn, hh, eng):
                _, spl, (sro, sco), oc = FOLD[pn]
                pt = fold_pt[(pn, hh)]
                pt3 = pt.rearrange("p (r q) -> p r q", q=64)
                ssl = spl[:, 32 * hh + sro:32 * hh + sro + 32, sco:sco + 64]
                ot = outp.tile([128, 2048], f16, tag="O")
                ot3 = ot.rearrange("p (r q) -> p r q", q=64)
                if eng == "dve":
                    nc.vector.tensor_tensor(out=ot3[:], in0=pt3[:], in1=ssl,
                                            op=MULT)
                else:
                    pf = scrp.tile([128, 2048], f16, tag="pf")
                    nc.scalar.copy(pf[:], pt[:, 0:2048])
                    pf3 = pf.rearrange("p (r q) -> p r q", q=64)
                    nc.gpsimd.tensor_tensor(out=ot3[:], in0=pf3[:],
                                            in1=ssl, op=MULT)
                nc.sync.dma_start(out_d[:, oc + 2048 * hh:
                                        oc + 2048 * hh + 2048], ot[:])

            # ---- the schedule ----
            if not add_bq:
                for ci in range(5):
                    qconv_chunk(*q_chunks[ci])
                emit_fused(2, 2)                    # t8 (EE)
                prods = [emit_product(i, j) for (i, j) in OFFL]
                emit_exp(8)
                emit_accred(0, prods[0])
                emit_exp(0)
                emit_fused(0, 1)                    # t1 (EO)
                qconv_chunk(*q_chunks[5])
                emit_accred(2, prods[1])
                emit_exp(2)
                emit_fused(2, 1)                    # t7 (EO)
                qconv_chunk(*q_chunks[6])
                emit_accred(6, prods[2])
                emit_exp(6)
                for t in (8, 0, 2):
                    emit_diag(t)
                emit_fused(1, 0)                    # t3 (OE)
                emit_exp(1)
                emit_exp(7)
                emit_diag(6)
                emit_diag(1)
                emit_diag(7)
                emit_vconv()
                qconv_chunk(*q_chunks[7])
                qconv_chunk(*q_chunks[8])
                fold_mm("oo", 0)
                fold_mm("oo", 1)
                emit_fused(1, 2)                    # t5 (OE)
                emit_exp(3)
                emit_diag(3)
                fold_final("oo", 0, "dve")
                emit_fused(1, 1)                    # t4 (OO)
                emit_exp(5)
                emit_diag(5)
                fold_final("oo", 1, "dve")
                emit_exp(4)
                nc.sync.dma_start(evs_d[:, :], sm[:, 9:18])
                fold_mm("oe", 0)
                fold_mm("oe", 1)
                fold_final("oe", 0, "gp")
                fold_final("oe", 1, "gp")
                fold_mm("eo", 0)
                fold_mm("eo", 1)
                fold_final("eo", 0, "dve")
                fold_final("eo", 1, "dve")
                for hh in range(2):
                    v00h = vc3[:, 32 * hh:32 * hh + 32, 0:64]
                    sslh = sOO[:, 32 * hh:32 * hh + 32, 0:64]
                    ot = outp.tile([128, 2048], f16, tag="O", name="otee")
                    ot3 = ot.rearrange("p (r q) -> p r q", q=64)
                    nc.vector.scalar_tensor_tensor(
                        out=ot3[:], in0=v00h, scalar=sm[:, 13:14],
                        in1=sslh, op0=MULT, op1=MULT)
                    nc.sync.dma_start(out_d[:, 2048 * hh:2048 * hh + 2048],
                                      ot[:])
            else:
                for c0, csz in q_chunks:
                    qconv_chunk(c0, csz)
                emit_vconv()
                for (i, j, ra, rb) in [(0, 0, 0, 32), (0, 2, 0, 32),
                                       (0, 0, 32, 64), (0, 2, 32, 64),
                                       (2, 0, 0, 64), (2, 2, 0, 64),
                                       (0, 1, 0, 64), (2, 1, 0, 64),
                                       (1, 0, 0, 64), (1, 2, 0, 64),
                                       (1, 1, 0, 64)]:
                    emit_fused(i, j, ra, rb)
                nc.scalar.activation(sm[:, 9:18], sm[:, 0:9], Exp,
                                     bias=0.0, scale=1.0)
                nc.sync.dma_start(evs_d[:, :], sm[:, 9:18])
                for t in range(9):
                    emit_diag(t)
                for pn in ("oo", "oe", "eo", "ee"):
                    for hh in range(2):
                        fold_mm(pn, hh)
                        fold_final(pn, hh,
                                   "gp" if pn in ("oe", "eo") else "dve")

    nc.compile()
    return nc


def _get_program(add_bv: bool, add_bq: bool):
    key = (add_bv, add_bq)
    if key not in _prog_cache:
        _prog_cache[key] = _build(add_bv, add_bq)
    return _prog_cache[key]


def _quad_pack(slab):
    """[C, 129, 129] -> [C, 16641] quadrant-packed [EE|EO|OE|OO]."""
    C = slab.shape[0]
    return np.concatenate([
        slab[:, 0::2, 0::2].reshape(C, -1),
        slab[:, 0::2, 1::2].reshape(C, -1),
        slab[:, 1::2, 0::2].reshape(C, -1),
        slab[:, 1::2, 1::2].reshape(C, -1),
    ], axis=1)


def kernel(feat, src, Wq, bq, Wv, bv):
    from concourse.bass_utils import run_bass_kernel_spmd

    feat = np.asarray(feat, dtype=np.float32)
    src = np.asarray(src, dtype=np.float32)
    Wq = np.asarray(Wq, dtype=np.float32)
    bq = np.asarray(bq, dtype=np.float32)
    Wv = np.asarray(Wv, dtype=np.float32)
    bv = np.asarray(bv, dtype=np.float32)
    B, C, H, W = src.shape

    src16 = np.pad(src, ((0, 0), (0, 0), (1, 1), (1, 1))).astype(np.float16)
    feat16 = np.pad(feat, ((0, 0), (0, 0), (0, 1), (0, 2))).astype(np.float16)
    eye = np.eye(128, dtype=np.float16)

    add_bv = bool(np.any(bv))
    have_bq = bool(np.any(bq))
    nc = _get_program(add_bv, have_bq)
    zero_sinit = np.zeros((128, 9), np.float32)

    in_maps = []
    perms = []
    for core in range(N_CORES):
        b, g = core // 2, core % 2
        mine = slice(128 * g, 128 * g + 128)
        other = slice(128 * (1 - g), 128 * (1 - g) + 128)
        perm = np.r_[np.arange(128 * g, 128 * g + 128),
                     np.arange(128 * (1 - g), 128 * (1 - g) + 128)]
        perms.append((b, mine))
        src_slab = _quad_pack(src16[b][perm][:, :SRC_R, :SRC_C])
        feat_slab = feat16[b][perm].reshape(C, FEATN)
        wp = np.concatenate([Wq.T[:, mine], Wv.T[:, mine]], axis=1)[perm]
        if have_bq:
            k = feat[b, mine]
            corr = np.zeros((128, 9), np.float32)
            for i in range(3):
                for j in range(3):
                    valid = np.ones((64, 64), bool)
                    if i == 0:
                        valid[0, :] = False
                    if j == 0:
                        valid[:, 0] = False
                    corr[:, 3 * i + j] = \
                        bq[mine] * (k * valid).sum((1, 2)) * SCALE
            sinit = corr
        else:
            sinit = zero_sinit
        in_maps.append({
            "src": np.ascontiguousarray(src_slab),
            "feat": np.ascontiguousarray(feat_slab),
            "wpack": np.ascontiguousarray(wp.astype(np.float16)),
            "eye": eye,
            "s_init": sinit,
            "bv": bv[mine].reshape(128, 1),
        })

    res = run_bass_kernel_spmd(nc, in_maps, list(range(N_CORES)),
                               trace=TRACE, **TRACE_KW)
    LAST_RESULT[0] = res

    out = np.empty((B, C, H, W), np.float32)
    for core in range(N_CORES):
        b, mine = perms[core]
        r = 1.0 / res.results[core]["evs"].sum(axis=1)
        o = res.results[core]["out"].astype(np.float32).reshape(128, 4, 64, 64)
        o = o * r[:, None, None, None]
        out[b, mine, 0::2, 0::2] = o[:, 0]
        out[b, mine, 0::2, 1::2] = o[:, 1]
        out[b, mine, 1::2, 0::2] = o[:, 2]
        out[b, mine, 1::2, 1::2] = o[:, 3]
    return out


# revision 32
# speedup vs baseline: 1.0414x; 1.0125x over previous
"""Trainium2 Bass kernel for nn_CRAU (per-channel sparse attention).

Computation (per batch b, channel c):
  qc  = Wq @ src (1x1 conv; bq folded into the S-reduction seed)
  S[c,t] = sum_d unfold(qc)[c,t,d] * feat[c,d] * (1/64)      t in 3x3 window
  A   = softmax_t(S)
  vc  = Wv @ feat + bv (1x1 conv)
  out = fold(A outer vc) * src

Sharding: 8 cores = 4 batches x 2 output-channel groups of 128. The
attention is fully per-channel, so with channel-group sharding each core
owns the complete spatial reduction for its channels -- no collectives.
Each core's inputs are channel-permuted host-side (own group first) so the
SPMD program always works on partition rows 0..127.

Layout: all tensors f16 host-side; the padded src slab is packed as 4
row/col-parity quadrant planes [EE|EO|OE|OO] so the 9-offset q.k reduce,
the fold, and the final multiply all read unit-stride blocks. qc inherits
the quadrant layout from the matmul rhs. The fold runs on the PE as
diag(A_t) matmuls accumulating parity half-planes in PSUM; finals
(plane * src) are split between DVE (PSUM-direct) and GpSimd.
"""

import numpy as np

N_CORES = 8
SRC_R, SRC_C = 129, 129         # per-core padded src slab (full spatial)
FEAT_R, FEAT_C = 65, 66         # padded feat slab
# quadrant plane offsets in the packed src/qc layout
O_EE = 0
O_EO = O_EE + 65 * 65           # 4225
O_OE = O_EO + 65 * 64           # 8385
O_OO = O_OE + 64 * 65           # 12545
SRCN = O_OO + 64 * 64           # 16641
FEATN = FEAT_R * FEAT_C         # 4290
OUTN = 4 * 4096                 # 4 parity planes per channel
SCALE = 1.0 / 64.0

_prog_cache = {}
TRACE = False
TRACE_KW = {}
LAST_RESULT = [None]


def _build(add_bv: bool, add_bq: bool = False):
    import concourse.mybir as mybir
    import concourse.tile as tile
    from concourse import bacc
    from concourse.dve_ops import TENSOR_TENSOR_REDUCE

    f32 = mybir.dt.float32
    f16 = mybir.dt.float16
    ADD = mybir.AluOpType.add
    MULT = mybir.AluOpType.mult
    MAX = mybir.AluOpType.max
    AX = mybir.AxisListType.X
    Exp = mybir.ActivationFunctionType.Exp
    Copy = mybir.ActivationFunctionType.Copy

    nc = bacc.Bacc("TRN2", target_bir_lowering=False, debug=False,
                   num_devices=N_CORES)

    src_d = nc.dram_tensor("src", [256, SRCN], f16, kind="ExternalInput").ap()
    feat_d = nc.dram_tensor("feat", [256, FEATN], f16, kind="ExternalInput").ap()
    wpack_d = nc.dram_tensor("wpack", [256, 256], f16, kind="ExternalInput").ap()
    eye_d = nc.dram_tensor("eye", [128, 128], f16, kind="ExternalInput").ap()
    sinit_d = nc.dram_tensor("s_init", [128, 9], f32, kind="ExternalInput").ap()
    bv_d = nc.dram_tensor("bv", [128, 1], f32, kind="ExternalInput").ap()
    out_d = nc.dram_tensor("out", [128, OUTN], f16, kind="ExternalOutput").ap()
    evs_d = nc.dram_tensor("evs", [128, 9], f32, kind="ExternalOutput").ap()

    QCH = 2048
    q_chunks = []
    c0 = 0
    while c0 < SRCN:
        q_chunks.append((c0, min(QCH, SRCN - c0)))
        c0 += QCH

    with tile.TileContext(nc) as tc:
        with (
            tc.tile_pool(name="srcp", bufs=2) as srcp,
            tc.tile_pool(name="featp", bufs=2) as featp,
            tc.tile_pool(name="qcp", bufs=1) as qcp,
            tc.tile_pool(name="vcp", bufs=1) as vcp,
            tc.tile_pool(name="constp", bufs=2) as constp,
            tc.tile_pool(name="smp", bufs=1) as smp,
            tc.tile_pool(name="scrp", bufs=2) as scrp,
            tc.tile_pool(name="outp", bufs=4) as outp,
            tc.tile_pool(name="ps", bufs=2, space="PSUM") as ps,
        ):
            # ---- loads ----
            w_t = []
            for kt in range(2):
                wt = constp.tile([128, 256], f16, tag="w", name=f"w{kt}")
                nc.sync.dma_start(wt[:], wpack_d[128 * kt:128 * kt + 128, :])
                w_t.append(wt)
            eye = constp.tile([128, 128], f16, tag="eye")
            nc.sync.dma_start(eye[:], eye_d[:, :])

            # smalls: [0:9] S [9:18] Ev [29] sum [30] r | [32:41] sinit
            # [41:42] bv
            sm = smp.tile([128, 48], f32, tag="smalls")
            nc.scalar.dma_start(sm[:, 32:41], sinit_d[:, :])
            if add_bv:
                nc.scalar.dma_start(sm[:, 41:42], bv_d[:, :])

            src_t = [srcp.tile([128, SRCN], f16, tag="src", name=f"src{kt}")
                     for kt in range(2)]
            feat_t = [featp.tile([128, FEATN], f16, tag="feat",
                                 name=f"feat{kt}") for kt in range(2)]
            # single gpsimd DMA stream, ordered so the reduce-critical data
            # lands first: src chunks 0-2, then k rows (feat_t[0] top), then
            # the rest of src, then the remaining feat
            HALF_F = 33 * FEAT_C
            for ci, (c0, csz) in enumerate(q_chunks):
                for kt in range(2):
                    nc.gpsimd.dma_start(
                        src_t[kt][:, c0:c0 + csz],
                        src_d[128 * kt:128 * kt + 128, c0:c0 + csz])
                if ci == 2:
                    nc.gpsimd.dma_start(feat_t[0][:, 0:HALF_F],
                                        feat_d[0:128, 0:HALF_F])
                elif ci == 3:
                    nc.gpsimd.dma_start(feat_t[0][:, HALF_F:FEATN],
                                        feat_d[0:128, HALF_F:FEATN])
            nc.gpsimd.dma_start(feat_t[1][:], feat_d[128:256, :])

            # ---- q-conv (PE) + PSUM->SBUF f16 copies (ScalarE) ----
            qct = qcp.tile([128, SRCN], f16, tag="qc")
            qEE = qct[:, O_EE:O_EO].rearrange("p (r q) -> p r q", q=65)
            qEO = qct[:, O_EO:O_OE].rearrange("p (r q) -> p r q", q=64)
            qOE = qct[:, O_OE:O_OO].rearrange("p (r q) -> p r q", q=65)
            qOO = qct[:, O_OO:SRCN].rearrange("p (r q) -> p r q", q=64)
            k3 = feat_t[0].rearrange("p (r q) -> p r q", q=FEAT_C)
            k64 = k3[:, 0:64, 0:64]

            def qconv_chunk(c0, csz):
                pt = ps.tile([128, 2048], f32, tag="mm", name="pt")
                for kt in range(2):
                    for s0 in range(0, csz, 512):
                        ssz = min(512, csz - s0)
                        nc.tensor.matmul(
                            pt[:, s0:s0 + ssz],
                            lhsT=w_t[kt][:, 0:128],
                            rhs=src_t[kt][:, c0 + s0:c0 + s0 + ssz],
                            start=(kt == 0), stop=(kt == 1))
                nc.scalar.copy(qct[:, c0:c0 + csz], pt[:, 0:csz])

            # offloaded EE offsets: DVE 2x products + ScalarE accum-reduce.
            # (only when bq==0 -- the accum path has no seed slot)
            OFFL = [] if add_bq else [(0, 0), (0, 2), (2, 0)]

            def emit_product(i, j):
                prod = scrp.tile([128, 4096], f16, tag="prod", name="prod")
                in0 = qEE[:, (1 if i == 2 else 0):(65 if i == 2 else 64),
                          (1 if j == 2 else 0):(65 if j == 2 else 64)]
                prod3 = prod.rearrange("p (r q) -> p r q", q=64)
                nc.vector.tensor_tensor(out=prod3[:], in0=in0, in1=k64,
                                        op=MULT)
                return prod

            def emit_accred(t, prod):
                dump = scrp.tile([128, 4096], f16, tag="dump")
                nc.scalar.activation(dump[:], prod[:], Copy, bias=0.0,
                                     scale=SCALE, accum_out=sm[:, t:t + 1])

            done = {}

            def emit_fused(i, j, ra=0, rb=64):
                t = 3 * i + j
                rlo = (1 if i == 2 else 0) + ra
                nrow = rb - ra
                if i == 1:
                    plane = qOO if j == 1 else qOE
                else:
                    plane = qEO if j == 1 else qEE
                clo = 1 if j == 2 else 0
                in0 = plane[:, rlo:rlo + nrow, clo:clo + 64]
                seed = sm[:, 32 + t:33 + t] if t not in done \
                    else sm[:, t:t + 1]
                scr = scrp.tile([128, 4096], f16, tag="scr")
                scr3 = scr.rearrange("p (r q) -> p r q", q=64)
                nc.vector._custom_dve(
                    TENSOR_TENSOR_REDUCE,
                    out=scr3[:, 0:nrow, :], in0=in0, in1=k64[:, ra:rb, :],
                    s0=seed, s1=SCALE,
                    accum_out=sm[:, t:t + 1])
                done[t] = done.get(t, 0) + 1

            dg = constp.tile([128, 9 * 128], f16, tag="dg")

            def emit_exp(t):
                nc.scalar.activation(sm[:, 9 + t:10 + t], sm[:, t:t + 1],
                                     Exp, bias=0.0, scale=1.0)

            def emit_diag(t):
                nc.vector.tensor_scalar(
                    out=dg[:, 128 * t:128 * t + 128], in0=eye[:],
                    scalar1=sm[:, 9 + t:10 + t], scalar2=None, op0=MULT)

            vt = vcp.tile([128, FEATN], f16, tag="vc")

            def emit_vconv():
                for c0 in range(0, FEATN, 2048):
                    csz = min(2048, FEATN - c0)
                    pt = ps.tile([128, 2048], f32, tag="mm", name="ptv")
                    for kt in range(2):
                        for s0 in range(0, csz, 512):
                            ssz = min(512, csz - s0)
                            nc.tensor.matmul(
                                pt[:, s0:s0 + ssz],
                                lhsT=w_t[kt][:, 128:256],
                                rhs=feat_t[kt][:, c0 + s0:c0 + s0 + ssz],
                                start=(kt == 0), stop=(kt == 1))
                    if add_bv:
                        nc.vector.tensor_scalar(
                            out=vt[:, c0:c0 + csz], in0=pt[:, 0:csz],
                            scalar1=sm[:, 41:42], scalar2=None, op0=ADD)
                    else:
                        nc.scalar.copy(vt[:, c0:c0 + csz], pt[:, 0:csz])
                if add_bv:
                    v3m = vt.rearrange("p (r q) -> p r q", q=FEAT_C)
                    nc.gpsimd.memset(v3m[:, FEAT_R - 1, :], 0.0)
                    nc.gpsimd.memset(v3m[:, :, 64:66], 0.0)

            # ---- fold plumbing: unnormalized planes; host applies 1/sum ----
            vc3 = vt.rearrange("p (r q) -> p r q", q=FEAT_C)
            sEE = src_t[0][:, O_EE:O_EO].rearrange("p (r q) -> p r q", q=65)
            sEO = src_t[0][:, O_EO:O_OE].rearrange("p (r q) -> p r q", q=64)
            sOE = src_t[0][:, O_OE:O_OO].rearrange("p (r q) -> p r q", q=65)
            sOO = src_t[0][:, O_OO:SRCN].rearrange("p (r q) -> p r q", q=64)
            # plane id -> (terms [(t, dr, dc)], src plane, src shift, out col)
            FOLD = {
                "ee": ([(4, 0, 0)], sOO, (0, 0), 0),
                "eo": ([(3, 0, 1), (5, 0, 0)], sOE, (0, 1), 4096),
                "oe": ([(1, 1, 0), (7, 0, 0)], sEO, (1, 0), 8192),
                "oo": ([(0, 1, 1), (2, 1, 0), (6, 0, 1), (8, 0, 0)],
                       sEE, (1, 1), 12288),
            }
            fold_pt = {}

            def fold_mm(pn, hh):
                terms, _, _, _ = FOLD[pn]
                pt = ps.tile([128, 2048], f32, tag="mm", name="ptf")
                pt3 = pt.rearrange("p (r q) -> p r q", q=64)
                nterm = len(terms)
                for n, (t, dr, dc) in enumerate(terms):
                    for s0 in range(4):
                        r0 = 32 * hh + dr + 8 * s0
                        nc.tensor.matmul(
                            pt3[:, 8 * s0:8 * s0 + 8, :],
                            lhsT=dg[:, 128 * t:128 * t + 128],
                            rhs=vc3[:, r0:r0 + 8, dc:dc + 64],
                            start=(n == 0), stop=(n == nterm - 1))
                fold_pt[(pn, hh)] = pt

            def fold_final(pn, hh, eng):
                _, spl, (sro, sco), oc = FOLD[pn]
                pt = fold_pt[(pn, hh)]
                pt3 = pt.rearrange("p (r q) -> p r q", q=64)
                ssl = spl[:, 32 * hh + sro:32 * hh + sro + 32, sco:sco + 64]
                ot = outp.tile([128, 2048], f16, tag="O")
                ot3 = ot.rearrange("p (r q) -> p r q", q=64)
                if eng == "dve":
                    nc.vector.tensor_tensor(out=ot3[:], in0=pt3[:], in1=ssl,
                                            op=MULT)
                else:
                    pf = scrp.tile([128, 2048], f16, tag="pf")
                    nc.scalar.copy(pf[:], pt[:, 0:2048])
                    pf3 = pf.rearrange("p (r q) -> p r q", q=64)
                    nc.gpsimd.tensor_tensor(out=ot3[:], in0=pf3[:],
                                            in1=ssl, op=MULT)
                nc.sync.dma_start(out_d[:, oc + 2048 * hh:
                                        oc + 2048 * hh + 2048], ot[:])

            # ---- the schedule ----
            if not add_bq:
                for ci in range(5):
                    qconv_chunk(*q_chunks[ci])
                emit_fused(2, 2)                    # t8 (EE)
                prods = [emit_product(i, j) for (i, j) in OFFL]
                emit_exp(8)
                emit_accred(0, prods[0])
                emit_exp(0)
                emit_fused(0, 1)                    # t1 (EO)
                qconv_chunk(*q_chunks[5])
                emit_accred(2, prods[1])
                emit_exp(2)
                emit_fused(2, 1)                    # t7 (EO)
                qconv_chunk(*q_chunks[6])
                emit_accred(6, prods[2])
                emit_exp(6)
                for t in (8, 0, 2):
                    emit_diag(t)
                emit_fused(1, 0)                    # t3 (OE)
                emit_exp(1)
                emit_exp(7)
                emit_diag(6)
                emit_diag(1)
                emit_diag(7)
                emit_vconv()
                qconv_chunk(*q_chunks[7])
                qconv_chunk(*q_chunks[8])
                fold_mm("oo", 0)
                fold_mm("oo", 1)
                emit_fused(1, 2)                    # t5 (OE)
                emit_fused(1, 1)                    # t4 (OO)
                emit_exp(3)
                emit_diag(3)
                emit_exp(5)
                emit_diag(5)
                fold_final("oo", 0, "dve")
                fold_final("oo", 1, "dve")
                emit_exp(4)
                nc.sync.dma_start(evs_d[:, :], sm[:, 9:18])
                fold_mm("oe", 0)
                fold_mm("oe", 1)
                fold_final("oe", 0, "gp")
                fold_final("oe", 1, "gp")
                fold_mm("eo", 0)
                fold_mm("eo", 1)
                fold_final("eo", 0, "dve")
                fold_final("eo", 1, "dve")
                for hh in range(2):
                    v00h = vc3[:, 32 * hh:32 * hh + 32, 0:64]
                    sslh = sOO[:, 32 * hh:32 * hh + 32, 0:64]
                    ot = outp.tile([128, 2048], f16, tag="O", name="otee")
                    ot3 = ot.rearrange("p (r q) -> p r q", q=64)
                    nc.vector.scalar_tensor_tensor(
                        out=ot3[:], in0=v00h, scalar=sm[:, 13:14],
                        in1=sslh, op0=MULT, op1=MULT)
                    nc.sync.dma_start(out_d[:, 2048 * hh:2048 * hh + 2048],
                                      ot[:])
            else:
                for c0, csz in q_chunks:
                    qconv_chunk(c0, csz)
                emit_vconv()
                for (i, j, ra, rb) in [(0, 0, 0, 32), (0, 2, 0, 32),
                                       (0, 0, 32, 64), (0, 2, 32, 64),
                                       (2, 0, 0, 64), (2, 2, 0, 64),
                                       (0, 1, 0, 64), (2, 1, 0, 64),
                                       (1, 0, 0, 64), (1, 2, 0, 64),
                                       (1, 1, 0, 64)]:
                    emit_fused(i, j, ra, rb)
                nc.scalar.activation(sm[:, 9:18], sm[:, 0:9], Exp,
                                     bias=0.0, scale=1.0)
                nc.sync.dma_start(evs_d[:, :], sm[:, 9:18])
                for t in range(9):
                    emit_diag(t)
                for pn in ("oo", "oe", "eo", "ee"):
                    for hh in range(2):
                        fold_mm(pn, hh)
                        fold_final(pn, hh,
                                   "gp" if pn in ("oe", "eo") else "dve")

    nc.compile()
    return nc


def _get_program(add_bv: bool, add_bq: bool):
    key = (add_bv, add_bq)
    if key not in _prog_cache:
        _prog_cache[key] = _build(add_bv, add_bq)
    return _prog_cache[key]


def _quad_pack(slab):
    """[C, 129, 129] -> [C, 16641] quadrant-packed [EE|EO|OE|OO]."""
    C = slab.shape[0]
    return np.concatenate([
        slab[:, 0::2, 0::2].reshape(C, -1),
        slab[:, 0::2, 1::2].reshape(C, -1),
        slab[:, 1::2, 0::2].reshape(C, -1),
        slab[:, 1::2, 1::2].reshape(C, -1),
    ], axis=1)


def kernel(feat, src, Wq, bq, Wv, bv):
    from concourse.bass_utils import run_bass_kernel_spmd

    feat = np.asarray(feat, dtype=np.float32)
    src = np.asarray(src, dtype=np.float32)
    Wq = np.asarray(Wq, dtype=np.float32)
    bq = np.asarray(bq, dtype=np.float32)
    Wv = np.asarray(Wv, dtype=np.float32)
    bv = np.asarray(bv, dtype=np.float32)
    B, C, H, W = src.shape

    src16 = np.pad(src, ((0, 0), (0, 0), (1, 1), (1, 1))).astype(np.float16)
    feat16 = np.pad(feat, ((0, 0), (0, 0), (0, 1), (0, 2))).astype(np.float16)
    eye = np.eye(128, dtype=np.float16)

    add_bv = bool(np.any(bv))
    have_bq = bool(np.any(bq))
    nc = _get_program(add_bv, have_bq)
    zero_sinit = np.zeros((128, 9), np.float32)

    in_maps = []
    perms = []
    for core in range(N_CORES):
        b, g = core // 2, core % 2
        mine = slice(128 * g, 128 * g + 128)
        other = slice(128 * (1 - g), 128 * (1 - g) + 128)
        perm = np.r_[np.arange(128 * g, 128 * g + 128),
                     np.arange(128 * (1 - g), 128 * (1 - g) + 128)]
        perms.append((b, mine))
        src_slab = _quad_pack(src16[b][perm][:, :SRC_R, :SRC_C])
        feat_slab = feat16[b][perm].reshape(C, FEATN)
        wp = np.concatenate([Wq.T[:, mine], Wv.T[:, mine]], axis=1)[perm]
        if have_bq:
            k = feat[b, mine]
            corr = np.zeros((128, 9), np.float32)
            for i in range(3):
                for j in range(3):
                    valid = np.ones((64, 64), bool)
                    if i == 0:
                        valid[0, :] = False
                    if j == 0:
                        valid[:, 0] = False
                    corr[:, 3 * i + j] = \
                        bq[mine] * (k * valid).sum((1, 2)) * SCALE
            sinit = corr
        else:
            sinit = zero_sinit
        in_maps.append({
            "src": np.ascontiguousarray(src_slab),
            "feat": np.ascontiguousarray(feat_slab),
            "wpack": np.ascontiguousarray(wp.astype(np.float16)),
            "eye": eye,
            "s_init": sinit,
            "bv": bv[mine].reshape(128, 1),
        })

    res = run_bass_kernel_spmd(nc, in_maps, list(range(N_CORES)),
                               trace=TRACE, **TRACE_KW)
    LAST_RESULT[0] = res

    out = np.empty((B, C, H, W), np.float32)
    for core in range(N_CORES):
        b, mine = perms[core]
        r = 1.0 / res.results[core]["evs"].sum(axis=1)
        o = res.results[core]["out"].astype(np.float32).reshape(128, 4, 64, 64)
        o = o * r[:, None, None, None]
        out[b, mine, 0::2, 0::2] = o[:, 0]
        out[b, mine, 0::2, 1::2] = o[:, 1]
        out[b, mine, 1::2, 0::2] = o[:, 2]
        out[b, mine, 1::2, 1::2] = o[:, 3]
    return out
